# revision 1
# baseline (speedup 1.0000x reference)
"""EpsSupInfoNCE loss on 8 Trainium2 NeuronCores.

Math (reference): logits = (E @ E.T)/temp;  same[i,j] = (label_i == label_j)
  S_j   = sum_i exp(logits[i,j]) * (1 - same[i,j])     (masked column sums)
  ce_ij = log(exp(-eps) + S_j * exp(-logits[i,j]))     for same-label i != j
  loss  = sum_j (1/count_j) * sum_i ce_ij / B

Strategy: columns are sharded over 8 cores; the HOST sorts all columns by
label first. Core c owns 1024 sorted columns whose same-label rows then
live in ONE contiguous row interval, padded to a 2048-row "window" (max
span ~1182 for this seed; LNW=1280 covers it). Rows are passed to each
core pre-split into et_win [D,2048] and et_main [D,6144] (order of rows
is irrelevant for the sums), which keeps the program SPMD-identical
across cores while the actual window offset varies per core. All
embedding inputs are bf16 (host-converted): no fp32r staging copies,
half the DMA bytes, and the first matmul starts as soon as its tile
lands. The scalar engine (ACT) is the bottleneck (every logit passes
through exp); its fused per-instruction accumulator (182ns readout) is
the cheapest column-sum by far (a DVE reduce of the same group costs
2.6us), so ACT keeps all S/A accumulation fused, and outputs stay f32
(bf16 ACT output measures ~20% slower).

Per 128-column tile (main groups first so ACT starts early; ct0's first
group is split 512+1536 so the very first EXP only waits on two tiny
DMAs):
  main rows:   logits matmul (bf16, 512-wide) -> PSUM; ACT exp(+l/temp)
               with fused accum_out -> S partials.
  window rows: logits matmul + one-hot mask matmul (-4.5 -> -C in logit
               units) -> PSUM; ACT exp(l/temp - C*same) -> Pwin, whose
               fused accum IS the masked S_win partial; DVE reciprocal
               R = 1/Pwin; ACT Ln(m_j * R + 1) with accum_out -> A_j
               partials, m_j = S_j*e^(eps-C): equals ce+eps at same-label
               entries, ~1e-17 at different-label ones.
Host: numer_sum_j = A_j - eps*count_j - log1p(S_j e^(eps-l_jj)) (the
diagonal term, which carries its own +eps), then a tiny f64 reduction.
Out-of-window same-label terms do not exist; out-of-window Ln terms are
< 1e-13 and are dropped.
"""
import numpy as np
import ml_dtypes
from contextlib import ExitStack

import concourse.bacc as bacc
import concourse.tile as tile
from concourse import mybir
from concourse.bass_utils import run_bass_kernel_spmd

B, D = 8192, 128
NCLS = 100
NCORES = 8
COLS = B // NCORES            # 1024 columns per core
NCT = COLS // 128             # 8 col-tiles per core
WIN = 2048                    # window rows per core
MAIN = B - WIN                # 6144 main rows per core
GROUP = 2048                  # rows per PSUM group (4 banks)
NGM = MAIN // GROUP           # 3 main groups
NSUB = GROUP // 512
LNW = 1280                    # Ln/recip slice: true same-label block size
FIRST = 512                   # ct0's tiny first chunk (early ACT start)

TEMP = 0.07
EPS = 0.25
SCALE = float(np.float32(1.0) / np.float32(TEMP))   # exp scale (fp32 value)
MASKVAL = -4.5                                      # bf16-exact additive mask
C_USED = 4.5 * SCALE                                # mask size in logit units
MCONST = float(np.exp(EPS - C_USED))                # e^(eps-C)

_cache = {}


def _patch_act_tables():
    """Steer Exp and Ln onto the one table set holding both, so Exp/Ln
    alternation doesn't thrash ACT_TABLE_LOADs. Set ids are indices into
    act_info.json, so keep dict length/order and just hide exp/ln
    elsewhere."""
    import concourse.hw_specs as hw_specs
    from concourse import mybir as _mb
    if getattr(bacc, "_act_tables_patched", False):
        return
    orig = hw_specs.get_activation_tables

    def steer(arch):
        t = orig(arch)
        exp, ln = (_mb.ActivationFunctionType.Exp, _mb.ActivationFunctionType.Ln)
        if "natural_log_exp_and_others" not in t:
            return t
        return {k: (fns if k == "natural_log_exp_and_others"
                    else fns - {exp, ln}) for k, fns in t.items()}

    bacc.get_activation_tables = steer
    bacc._act_tables_patched = True


def _build():
    dt = mybir.dt
    _patch_act_tables()
    nc = bacc.Bacc("TRN2", target_bir_lowering=False, debug=False,
                   num_devices=NCORES)
    et_main = nc.dram_tensor("et_main", [D, MAIN], dt.bfloat16,
                             kind="ExternalInput").ap()
    et_win = nc.dram_tensor("et_win", [D, WIN], dt.bfloat16,
                            kind="ExternalInput").ap()
    et_own = nc.dram_tensor("et_own", [D, COLS], dt.bfloat16,
                            kind="ExternalInput").ap()
    oh_win = nc.dram_tensor("oh_win", [NCLS, WIN], dt.bfloat16,
                            kind="ExternalInput").ap()
    ohn_own = nc.dram_tensor("ohn_own", [NCLS, COLS], dt.bfloat16,
                             kind="ExternalInput").ap()
    NSLOT = NGM + 1                 # S slots per col-tile (3 main + 1 win)
    NS = 1 + NCT * NSLOT            # col 0: ct0's extra FIRST-chunk slot
    out = nc.dram_tensor("out", [128, NS + NCT], dt.float32,
                         kind="ExternalOutput").ap()

    with tile.TileContext(nc) as tc:
        with ExitStack() as ctx:
            const_pool = ctx.enter_context(tc.tile_pool(name="consts", bufs=1))
            p_pool = ctx.enter_context(tc.tile_pool(name="pwin", bufs=2))
            r_pool = ctx.enter_context(tc.tile_pool(name="rbuf", bufs=2))
            d_pool = ctx.enter_context(tc.tile_pool(name="dump", bufs=2))
            ps_pool = ctx.enter_context(
                tc.tile_pool(name="psum", bufs=2, space="PSUM"))

            # Direct bf16 DMAs, two queues in parallel, ordered by first
            # consumption: ct0 runs main chunks first (the 512-col head
            # chunk needs only ~190KB of DMA), then the window.
            t_et_own = const_pool.tile([D, COLS], dt.bfloat16)
            t_et_main = const_pool.tile([D, MAIN], dt.bfloat16)
            t_et_win = const_pool.tile([D, WIN], dt.bfloat16)
            t_oh_win = const_pool.tile([NCLS, WIN], dt.bfloat16)
            t_ohn_own = const_pool.tile([NCLS, COLS], dt.bfloat16)
            # Coarse chunks (few DMAs: per-descriptor dispatch cost is
            # high, especially on the gpsimd software-DGE queue), ordered
            # by first consumption: ct0 needs et_own[:,:128] + et_main
            # immediately, the window set by ACT instr #5 (~10us in), and
            # the ct1..7-only slices (own/ohn cols 128:) last.
            nc.sync.dma_start(t_et_own[:, 0:128], et_own[:, 0:128])
            nc.sync.dma_start(t_et_main[:, 0:FIRST], et_main[:, 0:FIRST])
            nc.gpsimd.dma_start(t_et_main[:, FIRST:GROUP],
                                et_main[:, FIRST:GROUP])
            nc.sync.dma_start(t_et_main[:, GROUP:GROUP + 1536],
                              et_main[:, GROUP:GROUP + 1536])
            nc.gpsimd.dma_start(t_et_main[:, GROUP + 1536:2 * GROUP + 1024],
                                et_main[:, GROUP + 1536:2 * GROUP + 1024])
            nc.sync.dma_start(t_et_main[:, 2 * GROUP + 1024:MAIN],
                              et_main[:, 2 * GROUP + 1024:MAIN])
            nc.gpsimd.dma_start(t_et_win[:], et_win[:])
            nc.sync.dma_start(t_oh_win[:], oh_win[:])
            nc.gpsimd.dma_start(t_ohn_own[:, 0:128], ohn_own[:, 0:128])
            nc.sync.dma_start(t_et_own[:, 128:], et_own[:, 128:])
            nc.gpsimd.dma_start(t_ohn_own[:, 128:], ohn_own[:, 128:])

            s_part = const_pool.tile([128, NS], dt.float32)
            a_part = const_pool.tile([128, NCT], dt.float32)
            m_raw = const_pool.tile([128, NCT], dt.float32)
            m_sb = const_pool.tile([128, NCT], dt.float32)

            def emit_ln(ct, R):
                # ce-sum: Ln(m_j / Pwin + 1), fused per-column accumulate.
                dump = d_pool.tile([128, LNW], dt.float32, tag="dump2")
                nc.scalar.activation(
                    dump[:], R[:], mybir.ActivationFunctionType.Ln,
                    scale=m_sb[:, ct:ct + 1], bias=1.0,
                    accum_out=a_part[:, ct:ct + 1])

            def main_group(lhs_et, rows_lo, width, slot):
                ps = ps_pool.tile([128, GROUP], dt.float32, tag="ps")
                for n in range(width // 512):
                    nc.tensor.matmul(
                        ps[:, n * 512:(n + 1) * 512], lhs_et,
                        t_et_main[:, rows_lo + n * 512:
                                  rows_lo + (n + 1) * 512],
                        start=True, stop=True)
                dump = d_pool.tile([128, GROUP], dt.float32, tag="dump")
                nc.scalar.activation(
                    dump[:, 0:width], ps[:, 0:width],
                    mybir.ActivationFunctionType.Exp,
                    scale=SCALE, accum_out=s_part[:, slot:slot + 1])

            prev = None          # (ct, R) whose Ln is deferred one col-tile
            for ct in range(NCT):
                lhs_et = t_et_own[:, ct * 128:(ct + 1) * 128]
                lhs_oh = t_ohn_own[:, ct * 128:(ct + 1) * 128]
                base = 1 + ct * NSLOT

                # ---- main rows: unmasked, only feed S ----
                if ct == 0:
                    # split g0 so the first EXP waits on ~190KB of DMA
                    main_group(lhs_et, 0, FIRST, 0)
                    main_group(lhs_et, FIRST, GROUP - FIRST, base + 0)
                else:
                    main_group(lhs_et, 0 * GROUP, GROUP, base + 0)
                main_group(lhs_et, 1 * GROUP, GROUP, base + 1)
                main_group(lhs_et, 2 * GROUP, GROUP, base + 2)

                # ---- window rows: masked; feed S and the ce sum ----
                ps = ps_pool.tile([128, GROUP], dt.float32, tag="ps")
                for n in range(NSUB):
                    nc.tensor.matmul(
                        ps[:, n * 512:(n + 1) * 512], lhs_et,
                        t_et_win[:, n * 512:(n + 1) * 512],
                        start=True, stop=False)
                for n in range(NSUB):
                    nc.tensor.matmul(
                        ps[:, n * 512:(n + 1) * 512], lhs_oh,
                        t_oh_win[:, n * 512:(n + 1) * 512],
                        start=False, stop=True)
                # Pwin = exp(l - C*same); its fused accum IS the masked
                # S_win. The Ln input comes from the DVE reciprocal.
                P = p_pool.tile([128, WIN], dt.float32, tag="P")
                slot = base + NGM
                nc.scalar.activation(
                    P[:], ps[:], mybir.ActivationFunctionType.Exp,
                    scale=SCALE, accum_out=s_part[:, slot:slot + 1])
                # Same-label rows sit at window offset 0 (host layout), so
                # the numerator path only needs the first LNW rows.
                R = r_pool.tile([128, LNW], dt.float32, tag="R")
                nc.vector.reciprocal_approx_fast(out=R[:], in_=P[:, 0:LNW])

                lo = 0 if ct == 0 else base     # ct0's m includes slot 0
                nc.vector.reduce_sum(
                    m_raw[:, ct:ct + 1], s_part[:, lo:base + NSLOT],
                    axis=mybir.AxisListType.X)
                nc.vector.tensor_scalar_mul(
                    m_sb[:, ct:ct + 1], m_raw[:, ct:ct + 1], MCONST)

                # Defer this tile's Ln so the ACT FIFO can run the next
                # tile's exps while the DVE S/m chain completes.
                if prev is not None:
                    emit_ln(*prev)
                prev = (ct, R)
            # s_part is complete after ct7's window accum read; ship it
            # while the two remaining Lns run so only a_part is left for
            # the tail.
            nc.sync.dma_start(out[:, 0:NS], s_part[:])
            emit_ln(*prev)
            nc.sync.dma_start(out[:, NS:], a_part[:])
    nc.compile()
    return nc


def _get_nc():
    if "nc" not in _cache:
        _cache["nc"] = _build()
    return _cache["nc"]


def _prepare(embeds, labels):
    embeds = np.ascontiguousarray(np.asarray(embeds, dtype=np.float32))
    labels_i = np.asarray(labels).astype(np.int64)
    assert embeds.shape == (B, D)

    # Sort columns (and rows -- it is the same axis) by label so each
    # core's same-label rows are contiguous.
    perm = np.argsort(labels_i, kind="stable")
    lab = labels_i[perm]
    emb = embeds[perm]

    et = np.ascontiguousarray(emb.T).astype(ml_dtypes.bfloat16)   # [D, B]
    oh = np.zeros((NCLS, B), dtype=ml_dtypes.bfloat16)
    oh[lab, np.arange(B)] = ml_dtypes.bfloat16(1.0)
    ohn = (oh.astype(np.float32) * np.float32(MASKVAL)).astype(ml_dtypes.bfloat16)

    # class start/end in sorted order
    starts = np.searchsorted(lab, np.arange(NCLS), side="left")
    ends = np.searchsorted(lab, np.arange(NCLS), side="right")

    in_maps = []

    for c in range(NCORES):
        lo, hi = c * COLS, (c + 1) * COLS
        r_lo = int(starts[lab[lo]])
        r_hi = int(ends[lab[hi - 1]])
        span = r_hi - r_lo
        assert span <= LNW, f"window overflow: {span}"
        fill = WIN - span
        after = np.arange(r_hi, min(B, r_hi + fill))
        need = fill - len(after)
        before = np.arange(r_lo - need, r_lo) if need > 0 else np.arange(0)
        win_rows = np.concatenate([np.arange(r_lo, r_hi), after, before])
        assert len(win_rows) == WIN
        main_mask = np.ones(B, dtype=bool)
        main_mask[win_rows] = False
        main_idx = np.nonzero(main_mask)[0]
        in_maps.append({
            "et_main": np.ascontiguousarray(et[:, main_idx]),
            "et_win": np.ascontiguousarray(et[:, win_rows]),
            "et_own": np.ascontiguousarray(et[:, lo:hi]),
            "oh_win": np.ascontiguousarray(oh[:, win_rows]),
            "ohn_own": np.ascontiguousarray(ohn[:, lo:hi]),
        })
    return in_maps, lab, emb


def _combine(results, lab, emb):
    NSLOT = NGM + 1
    NS = 1 + NCT * NSLOT
    S = np.empty(B, dtype=np.float64)
    A = np.empty(B, dtype=np.float64)
    for c in range(NCORES):
        o = results[c]["out"].astype(np.float64)
        s = o[:, 1:NS].reshape(128, NCT, NSLOT).sum(-1)           # [p, ct]
        a = o[:, NS:NS + NCT]                                     # [p, ct]
        S[c * COLS:(c + 1) * COLS] = s.T.reshape(-1)              # j = ct*128+p
        S[c * COLS:c * COLS + 128] += o[:, 0]                     # ct0 extra
        A[c * COLS:(c + 1) * COLS] = a.T.reshape(-1)

    counts = np.bincount(lab, minlength=NCLS)
    count_j = counts[lab].astype(np.float64) - 1.0
    l_jj = (emb.astype(np.float64) ** 2).sum(1) * SCALE
    # A_j = sum_{in_numer}(ce+eps) + (ce_jj+eps); u_jj = ce_jj + eps, so
    # numer = A_j - eps*count_j - u_jj.
    u_jj = np.log1p(S * np.exp(EPS - l_jj))
    numer = A - EPS * count_j - u_jj
    loss = (numer / count_j).sum() / B
    return np.asarray(loss, dtype=np.float32)


def kernel(embeds, labels):
    in_maps, lab, emb = _prepare(embeds, labels)
    nc = _get_nc()
    res = run_bass_kernel_spmd(nc, in_maps, list(range(NCORES)))
    return _combine(res.results, lab, emb)



# revision 9
# speedup vs baseline: 1.1331x; 1.1331x over previous
"""EpsSupInfoNCE loss on 8 Trainium2 NeuronCores.

Math (reference): logits = (E @ E.T)/temp;  same[i,j] = (label_i == label_j)
  S_j   = sum_i exp(logits[i,j]) * (1 - same[i,j])     (masked column sums)
  ce_ij = log(exp(-eps) + S_j * exp(-logits[i,j]))     for same-label i != j
  loss  = sum_j (1/count_j) * sum_i ce_ij / B

Strategy: columns are sharded over 8 cores; the HOST sorts all columns by
label first. Core c owns 1024 sorted columns whose same-label rows then
live in ONE contiguous row interval, padded to a 2048-row "window" (max
span ~1182 for this seed; LNW=1280 covers it). Rows are passed to each
core pre-split into et_win [D,2048] and et_main [D,6144] (order of rows
is irrelevant for the sums), which keeps the program SPMD-identical
across cores while the actual window offset varies per core. All
embedding inputs are bf16 (host-converted): no fp32r staging copies,
half the DMA bytes, and the first matmul starts as soon as its tile
lands. The scalar engine (ACT) is the bottleneck (every logit passes
through exp); its fused per-instruction accumulator (182ns readout) is
the cheapest column-sum by far (a DVE reduce of the same group costs
2.6us), so ACT keeps all S/A accumulation fused, and outputs stay f32
(bf16 ACT output measures ~20% slower).

Per 128-column tile (main groups first so ACT starts early; ct0's first
group is split 512+1536 so the very first EXP only waits on two tiny
DMAs):
  main rows:   logits matmul (bf16, 512-wide) -> PSUM; ACT exp(+l/temp)
               with fused accum_out -> S partials.
  window rows: logits matmul + one-hot mask matmul (-4.5 -> -C in logit
               units) -> PSUM; ACT exp(l/temp - C*same) -> Pwin, whose
               fused accum IS the masked S_win partial; DVE reciprocal
               R = 1/Pwin; ACT Ln(m_j * R + 1) with accum_out -> A_j
               partials, m_j = S_j*e^(eps-C): equals ce+eps at same-label
               entries, ~1e-17 at different-label ones.
Host: numer_sum_j = A_j - eps*count_j - log1p(S_j e^(eps-l_jj)) (the
diagonal term, which carries its own +eps), then a tiny f64 reduction.
Out-of-window same-label terms do not exist; out-of-window Ln terms are
< 1e-13 and are dropped.
"""
import numpy as np
import ml_dtypes
from contextlib import ExitStack

import concourse.bacc as bacc
import concourse.tile as tile
from concourse import mybir
from concourse.bass_utils import run_bass_kernel_spmd

B, D = 8192, 128
NCLS = 100
NCORES = 8
COLS = B // NCORES            # 1024 columns per core
NCT = COLS // 128             # 8 col-tiles per core
WIN = 2048                    # window rows per core
MAIN = B - WIN                # 6144 main rows per core
GROUP = 2048                  # rows per PSUM group (4 banks)
NGM = MAIN // GROUP           # 3 main groups
NSUB = GROUP // 512
LNW = 1280                    # Ln/recip slice: true same-label block size
FIRST = 512                   # ct0's tiny first chunk (early ACT start)

TEMP = 0.07
EPS = 0.25
SCALE = float(np.float32(1.0) / np.float32(TEMP))   # exp scale (fp32 value)
MASKVAL = -4.5                                      # bf16-exact additive mask
C_USED = 4.5 * SCALE                                # mask size in logit units
MCONST = float(np.exp(EPS - C_USED))                # e^(eps-C)

_cache = {}


def _patch_act_tables():
    """Steer Exp and Ln onto the one table set holding both, so Exp/Ln
    alternation doesn't thrash ACT_TABLE_LOADs. Set ids are indices into
    act_info.json, so keep dict length/order and just hide exp/ln
    elsewhere."""
    import concourse.hw_specs as hw_specs
    from concourse import mybir as _mb
    if getattr(bacc, "_act_tables_patched", False):
        return
    orig = hw_specs.get_activation_tables

    def steer(arch):
        t = orig(arch)
        exp, ln = (_mb.ActivationFunctionType.Exp, _mb.ActivationFunctionType.Ln)
        if "natural_log_exp_and_others" not in t:
            return t
        return {k: (fns if k == "natural_log_exp_and_others"
                    else fns - {exp, ln}) for k, fns in t.items()}

    bacc.get_activation_tables = steer
    bacc._act_tables_patched = True


def _build():
    dt = mybir.dt
    _patch_act_tables()
    nc = bacc.Bacc("TRN2", target_bir_lowering=False, debug=False,
                   num_devices=NCORES)
    et_main = nc.dram_tensor("et_main", [D, MAIN], dt.bfloat16,
                             kind="ExternalInput").ap()
    et_win = nc.dram_tensor("et_win", [D, WIN], dt.bfloat16,
                            kind="ExternalInput").ap()
    et_own = nc.dram_tensor("et_own", [D, COLS], dt.bfloat16,
                            kind="ExternalInput").ap()
    oh_win = nc.dram_tensor("oh_win", [NCLS, WIN], dt.bfloat16,
                            kind="ExternalInput").ap()
    ohn_own = nc.dram_tensor("ohn_own", [NCLS, COLS], dt.bfloat16,
                             kind="ExternalInput").ap()
    NSLOT = NGM + 1                 # S slots per col-tile (3 main + 1 win)
    NS = 1 + NCT * NSLOT            # col 0: ct0's extra FIRST-chunk slot
    # out: s_part [128, NS] followed by per-col-tile P_win [128, LNW] f32.
    out = nc.dram_tensor("out", [128, NS + NCT * LNW], dt.float32,
                         kind="ExternalOutput").ap()

    with tile.TileContext(nc) as tc:
        with ExitStack() as ctx:
            const_pool = ctx.enter_context(tc.tile_pool(name="consts", bufs=1))
            p_pool = ctx.enter_context(tc.tile_pool(name="pwin", bufs=2))
            d_pool = ctx.enter_context(tc.tile_pool(name="dump", bufs=2))
            ps_pool = ctx.enter_context(
                tc.tile_pool(name="psum", bufs=2, space="PSUM"))

            # Direct bf16 DMAs, two queues in parallel, ordered by first
            # consumption: ct0 runs main chunks first (the 512-col head
            # chunk needs only ~190KB of DMA), then the window.
            t_et_own = const_pool.tile([D, COLS], dt.bfloat16)
            t_et_main = const_pool.tile([D, MAIN], dt.bfloat16)
            t_et_win = const_pool.tile([D, WIN], dt.bfloat16)
            t_oh_win = const_pool.tile([NCLS, WIN], dt.bfloat16)
            t_ohn_own = const_pool.tile([NCLS, COLS], dt.bfloat16)
            # Coarse chunks (few DMAs: per-descriptor dispatch cost is
            # high, especially on the gpsimd software-DGE queue), ordered
            # by first consumption: ct0 needs et_own[:,:128] + et_main
            # immediately, the window set by ACT instr #5 (~10us in), and
            # the ct1..7-only slices (own/ohn cols 128:) last.
            nc.sync.dma_start(t_et_own[:, 0:128], et_own[:, 0:128])
            nc.sync.dma_start(t_et_main[:, 0:FIRST], et_main[:, 0:FIRST])
            nc.gpsimd.dma_start(t_et_main[:, FIRST:GROUP],
                                et_main[:, FIRST:GROUP])
            nc.sync.dma_start(t_et_main[:, GROUP:GROUP + 1536],
                              et_main[:, GROUP:GROUP + 1536])
            nc.gpsimd.dma_start(t_et_main[:, GROUP + 1536:2 * GROUP + 1024],
                                et_main[:, GROUP + 1536:2 * GROUP + 1024])
            nc.sync.dma_start(t_et_main[:, 2 * GROUP + 1024:MAIN],
                              et_main[:, 2 * GROUP + 1024:MAIN])
            nc.gpsimd.dma_start(t_et_win[:], et_win[:])
            nc.sync.dma_start(t_oh_win[:], oh_win[:])
            nc.gpsimd.dma_start(t_ohn_own[:, 0:128], ohn_own[:, 0:128])
            nc.sync.dma_start(t_et_own[:, 128:], et_own[:, 128:])
            nc.gpsimd.dma_start(t_ohn_own[:, 128:], ohn_own[:, 128:])

            s_part = const_pool.tile([128, NS], dt.float32)

            def main_group(lhs_et, rows_lo, width, slot):
                ps = ps_pool.tile([128, GROUP], dt.float32, tag="ps")
                for n in range(width // 512):
                    nc.tensor.matmul(
                        ps[:, n * 512:(n + 1) * 512], lhs_et,
                        t_et_main[:, rows_lo + n * 512:
                                  rows_lo + (n + 1) * 512],
                        start=True, stop=True)
                dump = d_pool.tile([128, GROUP], dt.float32, tag="dump")
                nc.scalar.activation(
                    dump[:, 0:width], ps[:, 0:width],
                    mybir.ActivationFunctionType.Exp,
                    scale=SCALE, accum_out=s_part[:, slot:slot + 1])

            for ct in range(NCT):
                lhs_et = t_et_own[:, ct * 128:(ct + 1) * 128]
                lhs_oh = t_ohn_own[:, ct * 128:(ct + 1) * 128]
                base = 1 + ct * NSLOT

                # ---- main rows: unmasked, only feed S ----
                if ct == 0:
                    # split g0 so the first EXP waits on ~190KB of DMA
                    main_group(lhs_et, 0, FIRST, 0)
                    main_group(lhs_et, FIRST, GROUP - FIRST, base + 0)
                else:
                    main_group(lhs_et, 0 * GROUP, GROUP, base + 0)
                main_group(lhs_et, 1 * GROUP, GROUP, base + 1)
                main_group(lhs_et, 2 * GROUP, GROUP, base + 2)

                # ---- window rows: masked; feed S and the ce sum ----
                ps = ps_pool.tile([128, GROUP], dt.float32, tag="ps")
                for n in range(NSUB):
                    nc.tensor.matmul(
                        ps[:, n * 512:(n + 1) * 512], lhs_et,
                        t_et_win[:, n * 512:(n + 1) * 512],
                        start=True, stop=False)
                for n in range(NSUB):
                    nc.tensor.matmul(
                        ps[:, n * 512:(n + 1) * 512], lhs_oh,
                        t_oh_win[:, n * 512:(n + 1) * 512],
                        start=False, stop=True)
                # Pwin = exp(l - C*same); its fused accum IS the masked
                # S_win. The Ln input comes from the DVE reciprocal.
                P = p_pool.tile([128, WIN], dt.float32, tag="P")
                slot = base + NGM
                nc.scalar.activation(
                    P[:], ps[:], mybir.ActivationFunctionType.Exp,
                    scale=SCALE, accum_out=s_part[:, slot:slot + 1])
                # Same-label rows sit at window offset 0 (host layout), so
                # the numerator path only needs the first LNW rows. Ship
                # them raw; the host computes the ce/Ln terms.
                q = nc.sync if ct % 2 == 0 else nc.gpsimd
                q.dma_start(out[:, NS + ct * LNW:NS + (ct + 1) * LNW],
                            P[:, 0:LNW])
            nc.sync.dma_start(out[:, 0:NS], s_part[:])
    nc.compile()
    return nc


def _get_nc():
    if "nc" not in _cache:
        _cache["nc"] = _build()
    return _cache["nc"]


def _prepare(embeds, labels):
    embeds = np.ascontiguousarray(np.asarray(embeds, dtype=np.float32))
    labels_i = np.asarray(labels).astype(np.int64)
    assert embeds.shape == (B, D)

    # Sort columns (and rows -- it is the same axis) by label so each
    # core's same-label rows are contiguous.
    perm = np.argsort(labels_i, kind="stable")
    lab = labels_i[perm]
    emb = embeds[perm]

    et = np.ascontiguousarray(emb.T).astype(ml_dtypes.bfloat16)   # [D, B]
    oh = np.zeros((NCLS, B), dtype=ml_dtypes.bfloat16)
    oh[lab, np.arange(B)] = ml_dtypes.bfloat16(1.0)
    ohn = (oh.astype(np.float32) * np.float32(MASKVAL)).astype(ml_dtypes.bfloat16)

    # class start/end in sorted order
    starts = np.searchsorted(lab, np.arange(NCLS), side="left")
    ends = np.searchsorted(lab, np.arange(NCLS), side="right")

    in_maps = []
    win_rows_all = []

    for c in range(NCORES):
        lo, hi = c * COLS, (c + 1) * COLS
        r_lo = int(starts[lab[lo]])
        r_hi = int(ends[lab[hi - 1]])
        span = r_hi - r_lo
        assert span <= LNW, f"window overflow: {span}"
        fill = WIN - span
        after = np.arange(r_hi, min(B, r_hi + fill))
        need = fill - len(after)
        before = np.arange(r_lo - need, r_lo) if need > 0 else np.arange(0)
        win_rows = np.concatenate([np.arange(r_lo, r_hi), after, before])
        assert len(win_rows) == WIN
        win_rows_all.append(win_rows)
        main_mask = np.ones(B, dtype=bool)
        main_mask[win_rows] = False
        main_idx = np.nonzero(main_mask)[0]
        in_maps.append({
            "et_main": np.ascontiguousarray(et[:, main_idx]),
            "et_win": np.ascontiguousarray(et[:, win_rows]),
            "et_own": np.ascontiguousarray(et[:, lo:hi]),
            "oh_win": np.ascontiguousarray(oh[:, win_rows]),
            "ohn_own": np.ascontiguousarray(ohn[:, lo:hi]),
        })
    return in_maps, lab, win_rows_all


def _combine(results, lab, win_rows_all):
    NSLOT = NGM + 1
    NS = 1 + NCT * NSLOT
    S = np.empty(B, dtype=np.float64)
    for c in range(NCORES):
        o = results[c]["out"]
        s = o[:, 1:NS].astype(np.float64).reshape(128, NCT, NSLOT).sum(-1)
        S[c * COLS:(c + 1) * COLS] = s.T.reshape(-1)              # j = ct*128+p
        S[c * COLS:c * COLS + 128] += o[:, 0].astype(np.float64)  # ct0 extra

    logS = np.log(S)
    counts = np.bincount(lab, minlength=NCLS)
    count_j = counts[lab].astype(np.float64) - 1.0

    # numerator: ce_ij = log(e^-eps + S_j e^-l) at same-label i != j, with
    # l = ln(P_win) + C_USED recovered from the shipped masked exps.
    total = 0.0
    for c in range(NCORES):
        o = results[c]["out"]
        pw = o[:, NS:].reshape(128, NCT, LNW)                     # [p, ct, r]
        win = win_rows_all[c][:LNW]
        lab_win = lab[win]
        for ct in range(NCT):
            j_idx = c * COLS + ct * 128 + np.arange(128)
            m = (lab[j_idx][:, None] == lab_win[None, :]) \
                & (win[None, :] != j_idx[:, None])
            pj, rj = np.nonzero(m)
            l = np.log(pw[pj, ct, rj].astype(np.float64)) + C_USED
            jj = j_idx[pj]
            ce = np.logaddexp(-EPS, logS[jj] - l)
            total += (ce / count_j[jj]).sum()

    loss = total / B
    return np.asarray(loss, dtype=np.float32)


def kernel(embeds, labels):
    in_maps, lab, win_rows_all = _prepare(embeds, labels)
    nc = _get_nc()
    res = run_bass_kernel_spmd(nc, in_maps, list(range(NCORES)))
    return _combine(res.results, lab, win_rows_all)



# revision 31
# speedup vs baseline: 1.2480x; 1.1014x over previous
"""EpsSupInfoNCE loss on 8 Trainium2 NeuronCores — symmetry-dedup version.

logits = (E@E.T)/temp is SYMMETRIC, so each off-diagonal exp is computed
ONCE device-wide and feeds BOTH sums it belongs to: S for its column via
the ACT fused per-instruction accumulator, and S for its row via a
ones-vector matmul over P on the (underutilized) tensor engine.

Layout (host sorts all rows/cols by label first):
- core a owns cols [1024a, 1024(a+1)); 8 col-tiles of 128.
- window = rows [1024a, 1024a+2048) mod B  (own block + next block),
  MASKED via one-hot matmul (-4.5 in dot units -> -C in logit units).
  Own-block rows feed column-accum only (each within-block pair appears
  twice, once per mirror entry -> once per S side). Next-block rows feed
  column-accum AND a ones-matmul row-sum.
- main = sliding prefix of a circular band band[x] = 1024a+2048+x,
  x < 3072: col-tile k computes band[0:(17+k)*128]; rows [0:(16+k)*128]
  feed accum+ones; the last 128 rows (= tile u+32 for col-tile u) feed
  accum ONLY — its mirror tile does the same, covering diff-32 pairs
  once per side. All other cross-block tile pairs {u, v} are covered
  exactly once by the circular tournament v in {u+1..u+31}.
- numerator: P_win[:, 0:1280] shipped raw to DRAM; host recovers
  l = ln(P)+C at same-label entries (upper triangle, row>col) and
  computes both ordered ce terms from the single symmetric value.
- ones row-sums accumulate in persistent PSUM strips (2 banks, 8 slots
  of [1,512] at partitions 0/32/64/96) across all col-tiles; one DMA
  per slot at the end. PSUM: 2x 3-bank matmul groups + 2 banks = 8.
"""
import numpy as np
import ml_dtypes
from contextlib import ExitStack

import concourse.bacc as bacc
import concourse.tile as tile
from concourse import mybir
from concourse.bass_utils import run_bass_kernel_spmd

B, D = 8192, 128
NCLS = 100
NCORES = 8
COLS = B // NCORES            # 1024 columns per core
NCT = COLS // 128             # 8 col-tiles per core
WIN = 2048                    # window rows per core (blocks a, a+1)
BAND = 3072                   # circular main band rows per core
LNW = 1280                    # shipped window rows (numerator span)
G1 = 1024                     # PSUM group width (2 banks; ones need 3,
                              # PE PSUM col base is limited to {0,32,64})
FIRST = 512                   # m0's tiny first chunk (early ACT start)

TEMP = 0.07
EPS = 0.25
SCALE = float(np.float32(1.0) / np.float32(TEMP))   # exp scale (fp32 value)
MASKVAL = -4.5                                      # bf16-exact additive mask
C_USED = 4.5 * SCALE                                # mask size in logit units

# program order: window k ships 640KB of P, so keep the last mains after
# the last window to hide the final ship under trailing ACT work.
ORDER = ["m0", "m1", "m2", "w0", "m3", "w1", "m4", "w2", "m5", "w3",
         "w4", "w5", "w6", "w7", "m6", "m7"]

_cache = {}


def _patch_act_tables():
    """Steer Exp onto a stable table set (baseline's patch; harmless now
    that only Exp is used)."""
    import concourse.hw_specs as hw_specs
    from concourse import mybir as _mb
    if getattr(bacc, "_act_tables_patched", False):
        return
    orig = hw_specs.get_activation_tables

    def steer(arch):
        t = orig(arch)
        exp, ln = (_mb.ActivationFunctionType.Exp, _mb.ActivationFunctionType.Ln)
        if "natural_log_exp_and_others" not in t:
            return t
        return {k: (fns if k == "natural_log_exp_and_others"
                    else fns - {exp, ln}) for k, fns in t.items()}

    bacc.get_activation_tables = steer
    bacc._act_tables_patched = True


def _slots():
    """Deterministic accum-slot layout shared by _build and _combine.
    Returns (slot_of[step_name] -> list of slot ids, NS)."""
    slot_of = {}
    n = 0
    for step in ORDER:
        if step[0] == "m":
            nslots = 3 + (1 if step == "m0" else 0)
        else:
            nslots = 2
        slot_of[step] = list(range(n, n + nslots))
        n += nslots
    return slot_of, n


def _build():
    dt = mybir.dt
    _patch_act_tables()
    nc = bacc.Bacc("TRN2", target_bir_lowering=False, debug=False,
                   num_devices=NCORES)
    et_own = nc.dram_tensor("et_own", [D, COLS], dt.bfloat16,
                            kind="ExternalInput").ap()
    et_win = nc.dram_tensor("et_win", [D, WIN], dt.bfloat16,
                            kind="ExternalInput").ap()
    et_band = nc.dram_tensor("et_band", [D, BAND], dt.bfloat16,
                             kind="ExternalInput").ap()
    oh_win = nc.dram_tensor("oh_win", [NCLS, WIN], dt.bfloat16,
                            kind="ExternalInput").ap()
    ohn_own = nc.dram_tensor("ohn_own", [NCLS, COLS], dt.bfloat16,
                             kind="ExternalInput").ap()
    slot_of, NS = _slots()
    out = nc.dram_tensor("out", [128, NS], dt.float32,
                         kind="ExternalOutput").ap()
    # P is produced as float32r (PE consumes it as fp32r moving data);
    # same bits as f32 on the host side.
    pout = nc.dram_tensor("pout", [128, NCT * LNW], dt.float32r,
                          kind="ExternalOutput").ap()
    ones_out = nc.dram_tensor("ones_out", [1, 4096], dt.float32,
                              kind="ExternalOutput").ap()

    with tile.TileContext(nc) as tc:
        with ExitStack() as ctx:
            const_pool = ctx.enter_context(tc.tile_pool(name="consts", bufs=1))
            p_pool = ctx.enter_context(tc.tile_pool(name="pwin", bufs=2))
            d_pool = ctx.enter_context(tc.tile_pool(name="pmain", bufs=3))
            ps_pool = ctx.enter_context(
                tc.tile_pool(name="psum", bufs=2, space="PSUM"))
            po_pool = ctx.enter_context(
                tc.tile_pool(name="psones", bufs=2, space="PSUM"))

            t_et_own = const_pool.tile([D, COLS], dt.bfloat16)
            t_et_win = const_pool.tile([D, WIN], dt.bfloat16)
            t_et_band = const_pool.tile([D, BAND], dt.bfloat16)
            t_oh_win = const_pool.tile([NCLS, WIN], dt.bfloat16)
            t_ohn_own = const_pool.tile([NCLS, COLS], dt.bfloat16)
            # Ordered by first consumption (m0 needs own[0:128]+band head).
            nc.sync.dma_start(t_et_own[:, 0:128], et_own[:, 0:128])
            nc.sync.dma_start(t_et_band[:, 0:FIRST], et_band[:, 0:FIRST])
            nc.sync.dma_start(t_et_band[:, FIRST:G1], et_band[:, FIRST:G1])
            nc.gpsimd.dma_start(t_et_band[:, G1:2432], et_band[:, G1:2432])
            nc.sync.dma_start(t_et_win[:, 0:G1], et_win[:, 0:G1])
            nc.gpsimd.dma_start(t_et_win[:, G1:], et_win[:, G1:])
            nc.sync.dma_start(t_ohn_own[:, 0:128], ohn_own[:, 0:128])
            nc.gpsimd.dma_start(t_oh_win[:, 0:G1], oh_win[:, 0:G1])
            nc.gpsimd.dma_start(t_oh_win[:, G1:], oh_win[:, G1:])
            nc.sync.dma_start(t_et_band[:, 2432:], et_band[:, 2432:])
            nc.sync.dma_start(t_et_own[:, 128:], et_own[:, 128:])
            nc.gpsimd.dma_start(t_ohn_own[:, 128:], ohn_own[:, 128:])

            ones_f = const_pool.tile([128, 128], dt.float32)
            nc.gpsimd.memset(ones_f[:], 1.0)
            ones_t = const_pool.tile([128, 128], dt.float32r)
            nc.scalar.copy(ones_t[:], ones_f[:])   # legal fp32r producer
            ones_r = ones_t[:]

            s_part = const_pool.tile([128, NS], dt.float32)
            # Row-sums: ones[128,128] lhsT replicates each chunk's sums
            # across all 128 partitions (PE PSUM writes must start at
            # partition 0 for fp32r). Each step's chunks land in a fresh
            # 2-bank PSUM tile; one DVE add folds them into acc_sb.
            # Slot s of acc_sb covers free [512s, 512s+512).
            acc_sb = const_pool.tile([128, 4096], dt.float32)
            nc.gpsimd.memset(acc_sb[:], 0.0)

            def ones_block(pairs, acc_lo, acc_w):
                """pairs: list of (rhs_ap, width); summed row-chunks land
                at acc_sb[:, acc_lo:acc_lo+acc_w]."""
                po = po_pool.tile([128, G1], dt.float32, tag="po")
                off = 0
                for rhs, w in pairs:
                    nc.tensor.matmul(po[:, off:off + w], ones_r, rhs,
                                     start=True, stop=True)
                    off += w
                assert off == acc_w
                nc.vector.tensor_add(
                    acc_sb[:, acc_lo:acc_lo + acc_w],
                    acc_sb[:, acc_lo:acc_lo + acc_w], po[:, 0:acc_w])

            def group(lhs_et, rhs_t, r0, w, P, p_off, slot, lhs_oh=None,
                      oh_t=None, split=False):
                """One PSUM group: logit (+ mask) matmuls, then EXP with
                fused accum into s_part[slot]."""
                ps = ps_pool.tile([128, G1], dt.float32, tag="ps")
                off = 0
                while off < w:
                    cw = min(512, w - off)
                    nc.tensor.matmul(ps[:, off:off + cw], lhs_et,
                                     rhs_t[:, r0 + off:r0 + off + cw],
                                     start=True, stop=(lhs_oh is None))
                    off += cw
                if lhs_oh is not None:
                    off = 0
                    while off < w:
                        cw = min(512, w - off)
                        nc.tensor.matmul(ps[:, off:off + cw], lhs_oh,
                                         oh_t[:, r0 + off:r0 + off + cw],
                                         start=False, stop=True)
                        off += cw
                if split:
                    nc.scalar.activation(
                        P[:, p_off:p_off + FIRST], ps[:, 0:FIRST],
                        mybir.ActivationFunctionType.Exp,
                        scale=SCALE, accum_out=s_part[:, slot:slot + 1])
                    nc.scalar.activation(
                        P[:, p_off + FIRST:p_off + w], ps[:, FIRST:w],
                        mybir.ActivationFunctionType.Exp,
                        scale=SCALE, accum_out=s_part[:, slot + 1:slot + 2])
                else:
                    nc.scalar.activation(
                        P[:, p_off:p_off + w], ps[:, 0:w],
                        mybir.ActivationFunctionType.Exp,
                        scale=SCALE, accum_out=s_part[:, slot:slot + 1])

            def w_step(k):
                lhs_et = t_et_own[:, k * 128:(k + 1) * 128]
                lhs_oh = t_ohn_own[:, k * 128:(k + 1) * 128]
                sl = slot_of[f"w{k}"]
                P = p_pool.tile([128, WIN], dt.float32r, tag="P")
                # group W1: own-block rows [0:1024] — accum only
                group(lhs_et, t_et_win, 0, G1, P, 0, sl[0],
                      lhs_oh=lhs_oh, oh_t=t_oh_win)
                # group W2: FWD-block rows [1024:2048] — accum + ones
                group(lhs_et, t_et_win, G1, G1, P, G1, sl[1],
                      lhs_oh=lhs_oh, oh_t=t_oh_win)
                ones_block([(P[:, 1024:1536], 512), (P[:, 1536:2048], 512)],
                           3072, 1024)
                # ship numerator rows
                q = nc.sync if k % 2 == 0 else nc.gpsimd
                q.dma_start(pout[:, k * LNW:(k + 1) * LNW],
                            P[:, 0:LNW])

            def m_step(k):
                lhs_et = t_et_own[:, k * 128:(k + 1) * 128]
                sl = slot_of[f"m{k}"]
                n_main = (17 + k) * 128
                n_ones = (16 + k) * 128
                # groups over band rows [0:1024), [1024:2048), [2048:n_main)
                Pa = d_pool.tile([128, G1], dt.float32r, tag="Pm")
                group(lhs_et, t_et_band, 0, G1, Pa, 0, sl[0],
                      split=(k == 0))
                si = 2 if k == 0 else 1
                ones_block([(Pa[:, 0:512], 512), (Pa[:, 512:1024], 512)],
                           0, 1024)
                Pb = d_pool.tile([128, G1], dt.float32r, tag="Pm")
                group(lhs_et, t_et_band, G1, G1, Pb, 0, sl[si])
                ones_block([(Pb[:, 0:512], 512), (Pb[:, 512:1024], 512)],
                           1024, 1024)
                wd = n_main - 2048
                Pc = d_pool.tile([128, G1], dt.float32r, tag="Pm")
                group(lhs_et, t_et_band, 2048, wd, Pc, 0, sl[si + 1])
                # ones over band rows [2048:n_ones] (width 128k)
                w3 = n_ones - 2048
                if w3 > 0:
                    pairs = [(Pc[:, 0:min(512, w3)], min(512, w3))]
                    if w3 > 512:
                        pairs.append((Pc[:, 512:w3], w3 - 512))
                    ones_block(pairs, 2048, w3)

            for step in ORDER:
                (m_step if step[0] == "m" else w_step)(int(step[1]))

            nc.sync.dma_start(out[:], s_part[:])
            nc.gpsimd.dma_start(ones_out[:], acc_sb[0:1, :])
    nc.compile()
    return nc


def _get_nc():
    if "nc" not in _cache:
        _cache["nc"] = _build()
    return _cache["nc"]


def _prepare(embeds, labels):
    embeds = np.ascontiguousarray(np.asarray(embeds, dtype=np.float32))
    labels_i = np.asarray(labels).astype(np.int64)
    assert embeds.shape == (B, D)

    perm = np.argsort(labels_i, kind="stable")
    lab = labels_i[perm]
    emb = embeds[perm]

    et = np.ascontiguousarray(emb.T).astype(ml_dtypes.bfloat16)   # [D, B]
    oh = np.zeros((NCLS, B), dtype=ml_dtypes.bfloat16)
    oh[lab, np.arange(B)] = ml_dtypes.bfloat16(1.0)
    ohn = (oh.astype(np.float32) * np.float32(MASKVAL)).astype(
        ml_dtypes.bfloat16)

    in_maps = []
    for c in range(NCORES):
        lo = c * COLS
        win = (lo + np.arange(WIN)) % B
        band = (lo + WIN + np.arange(BAND)) % B
        r_hi = np.searchsorted(lab, lab[lo + COLS - 1], side="right")
        assert r_hi - lo <= LNW, f"window overflow: {r_hi - lo}"
        in_maps.append({
            "et_own": np.ascontiguousarray(et[:, lo:lo + COLS]),
            "et_win": np.ascontiguousarray(et[:, win]),
            "et_band": np.ascontiguousarray(et[:, band]),
            "oh_win": np.ascontiguousarray(oh[:, win]),
            "ohn_own": np.ascontiguousarray(ohn[:, lo:lo + COLS]),
        })
    return in_maps, lab


def _combine(results, lab):
    slot_of, NS = _slots()
    S = np.zeros(B, dtype=np.float64)
    for a in range(NCORES):
        o = results[a]["out"]
        oo = results[a]["ones_out"].astype(np.float64).reshape(-1)
        sp = o[:, 0:NS].astype(np.float64)
        # column accums: sum each col-tile's slots
        for k in range(NCT):
            cols = a * COLS + k * 128 + np.arange(128)
            acc = np.zeros(128)
            for st in (f"m{k}", f"w{k}"):
                for s in slot_of[st]:
                    acc += sp[:, s]
            S[cols] += acc
        # ones row-sums
        win = (a * COLS + np.arange(WIN)) % B
        band = (a * COLS + WIN + np.arange(BAND)) % B
        S[band] += oo[0:BAND]                   # main slots 0..5
        S[win[1024:2048]] += oo[3072:4096]      # window FWD slots 6,7

    counts = np.bincount(lab, minlength=NCLS)
    count_j = counts[lab].astype(np.float64) - 1.0
    logS = np.log(S)

    total = 0.0
    for a in range(NCORES):
        pw = results[a]["pout"].reshape(128, NCT, LNW)
        win = (a * COLS + np.arange(LNW)) % B
        lab_win = lab[win]
        for k in range(NCT):
            cols = a * COLS + k * 128 + np.arange(128)
            m = (lab[cols][:, None] == lab_win[None, :]) \
                & (win[None, :] > cols[:, None])
            pj, rj = np.nonzero(m)
            i_idx = cols[pj]
            j_idx = win[rj]
            l = np.log(pw[pj, k, rj].astype(np.float64)) + C_USED
            ce_ij = np.logaddexp(-EPS, logS[j_idx] - l)
            ce_ji = np.logaddexp(-EPS, logS[i_idx] - l)
            total += (ce_ij / count_j[j_idx]).sum() \
                + (ce_ji / count_j[i_idx]).sum()

    loss = total / B
    return np.asarray(loss, dtype=np.float32)


def kernel(embeds, labels):
    in_maps, lab = _prepare(embeds, labels)
    nc = _get_nc()
    res = run_bass_kernel_spmd(nc, in_maps, list(range(NCORES)))
    return _combine(res.results, lab)


# revision 34
# speedup vs baseline: 1.2675x; 1.0156x over previous
"""EpsSupInfoNCE loss on 8 Trainium2 NeuronCores — symmetry-dedup version.

logits = (E@E.T)/temp is SYMMETRIC, so each off-diagonal exp is computed
ONCE device-wide and feeds BOTH sums it belongs to: S for its column via
the ACT fused per-instruction accumulator, and S for its row via a
ones-vector matmul over P on the (underutilized) tensor engine.

Layout (host sorts all rows/cols by label first):
- core a owns cols [1024a, 1024(a+1)); 8 col-tiles of 128.
- window = rows [1024a, 1024a+2048) mod B  (own block + next block),
  MASKED via one-hot matmul (-4.5 in dot units -> -C in logit units).
  Own-block rows feed column-accum only (each within-block pair appears
  twice, once per mirror entry -> once per S side). Next-block rows feed
  column-accum AND a ones-matmul row-sum.
- main = sliding prefix of a circular band band[x] = 1024a+2048+x,
  x < 3072: col-tile k computes band[0:(17+k)*128]; rows [0:(16+k)*128]
  feed accum+ones; the last 128 rows (= tile u+32 for col-tile u) feed
  accum ONLY — its mirror tile does the same, covering diff-32 pairs
  once per side. All other cross-block tile pairs {u, v} are covered
  exactly once by the circular tournament v in {u+1..u+31}.
- numerator: P_win[:, 0:1280] shipped raw to DRAM; host recovers
  l = ln(P)+C at same-label entries (upper triangle, row>col) and
  computes both ordered ce terms from the single symmetric value.
- ones row-sums accumulate in persistent PSUM strips (2 banks, 8 slots
  of [1,512] at partitions 0/32/64/96) across all col-tiles; one DMA
  per slot at the end. PSUM: 2x 3-bank matmul groups + 2 banks = 8.
"""
import numpy as np
import ml_dtypes
from contextlib import ExitStack

import concourse.bacc as bacc
import concourse.tile as tile
from concourse import mybir
from concourse.bass_utils import run_bass_kernel_spmd

B, D = 8192, 128
NCLS = 100
NCORES = 8
COLS = B // NCORES            # 1024 columns per core
NCT = COLS // 128             # 8 col-tiles per core
WIN = 2048                    # window rows per core (blocks a, a+1)
BAND = 3072                   # circular main band rows per core
LNW = 1280                    # shipped window rows (numerator span)
G1 = 1024                     # PSUM group width (2 banks; ones need 3,
                              # PE PSUM col base is limited to {0,32,64})
FIRST = 512                   # m0's tiny first chunk (early ACT start)

TEMP = 0.07
EPS = 0.25
SCALE = float(np.float32(1.0) / np.float32(TEMP))   # exp scale (fp32 value)
MASKVAL = -4.5                                      # bf16-exact additive mask
C_USED = 4.5 * SCALE                                # mask size in logit units

# program order: window k ships 640KB of P, so keep the last mains after
# the last window to hide the final ship under trailing ACT work.
ORDER = ["m0", "m1", "m2", "w0", "m3", "w1", "m4", "w2", "m5", "w3",
         "w4", "w5", "w6", "w7", "m6", "m7"]

_cache = {}


def _patch_act_tables():
    """Steer Exp onto a stable table set (baseline's patch; harmless now
    that only Exp is used)."""
    import concourse.hw_specs as hw_specs
    from concourse import mybir as _mb
    if getattr(bacc, "_act_tables_patched", False):
        return
    orig = hw_specs.get_activation_tables

    def steer(arch):
        t = orig(arch)
        exp, ln = (_mb.ActivationFunctionType.Exp, _mb.ActivationFunctionType.Ln)
        if "natural_log_exp_and_others" not in t:
            return t
        return {k: (fns if k == "natural_log_exp_and_others"
                    else fns - {exp, ln}) for k, fns in t.items()}

    bacc.get_activation_tables = steer
    bacc._act_tables_patched = True


def _slots():
    """Deterministic accum-slot layout shared by _build and _combine.
    Returns (slot_of[step_name] -> list of slot ids, NS)."""
    slot_of = {}
    n = 0
    for step in ORDER:
        if step[0] == "m":
            nslots = 3 + (1 if step == "m0" else 0)
        else:
            nslots = 2
        slot_of[step] = list(range(n, n + nslots))
        n += nslots
    return slot_of, n


def _build():
    dt = mybir.dt
    _patch_act_tables()
    nc = bacc.Bacc("TRN2", target_bir_lowering=False, debug=False,
                   num_devices=NCORES)
    et_own = nc.dram_tensor("et_own", [D, COLS], dt.bfloat16,
                            kind="ExternalInput").ap()
    et_win = nc.dram_tensor("et_win", [D, WIN], dt.bfloat16,
                            kind="ExternalInput").ap()
    et_band = nc.dram_tensor("et_band", [D, BAND], dt.bfloat16,
                             kind="ExternalInput").ap()
    oh_win = nc.dram_tensor("oh_win", [NCLS, WIN], dt.bfloat16,
                            kind="ExternalInput").ap()
    ohn_own = nc.dram_tensor("ohn_own", [NCLS, COLS], dt.bfloat16,
                             kind="ExternalInput").ap()
    slot_of, NS = _slots()
    out = nc.dram_tensor("out", [128, NS], dt.float32,
                         kind="ExternalOutput").ap()
    # P is produced as float32r (PE consumes it as fp32r moving data);
    # same bits as f32 on the host side.
    pout = nc.dram_tensor("pout", [128, NCT * LNW], dt.float32r,
                          kind="ExternalOutput").ap()
    ones_out = nc.dram_tensor("ones_out", [1, 4096], dt.float32,
                              kind="ExternalOutput").ap()

    with tile.TileContext(nc) as tc:
        with ExitStack() as ctx:
            const_pool = ctx.enter_context(tc.tile_pool(name="consts", bufs=1))
            p_pool = ctx.enter_context(tc.tile_pool(name="pwin", bufs=2))
            d_pool = ctx.enter_context(tc.tile_pool(name="pmain", bufs=3))
            ps_pool = ctx.enter_context(
                tc.tile_pool(name="psum", bufs=2, space="PSUM"))
            po_pool = ctx.enter_context(
                tc.tile_pool(name="psones", bufs=2, space="PSUM"))

            t_et_own = const_pool.tile([D, COLS], dt.bfloat16)
            t_et_win = const_pool.tile([D, WIN], dt.bfloat16)
            t_et_band = const_pool.tile([D, BAND], dt.bfloat16)
            t_oh_win = const_pool.tile([NCLS, WIN], dt.bfloat16)
            t_ohn_own = const_pool.tile([NCLS, COLS], dt.bfloat16)
            # Ordered by first consumption (m0 needs own[0:128]+band head).
            nc.sync.dma_start(t_et_own[:, 0:128], et_own[:, 0:128])
            nc.sync.dma_start(t_et_band[:, 0:FIRST], et_band[:, 0:FIRST])
            nc.sync.dma_start(t_et_band[:, FIRST:G1], et_band[:, FIRST:G1])
            nc.gpsimd.dma_start(t_et_band[:, G1:2432], et_band[:, G1:2432])
            nc.sync.dma_start(t_et_win[:, 0:G1], et_win[:, 0:G1])
            nc.gpsimd.dma_start(t_et_win[:, G1:], et_win[:, G1:])
            nc.sync.dma_start(t_ohn_own[:, 0:128], ohn_own[:, 0:128])
            nc.gpsimd.dma_start(t_oh_win[:, 0:G1], oh_win[:, 0:G1])
            nc.gpsimd.dma_start(t_oh_win[:, G1:], oh_win[:, G1:])
            nc.sync.dma_start(t_et_band[:, 2432:], et_band[:, 2432:])
            nc.sync.dma_start(t_et_own[:, 128:], et_own[:, 128:])
            nc.gpsimd.dma_start(t_ohn_own[:, 128:], ohn_own[:, 128:])

            ones_f = const_pool.tile([128, 128], dt.float32)
            nc.gpsimd.memset(ones_f[:], 1.0)
            ones_t = const_pool.tile([128, 128], dt.float32r)
            nc.scalar.copy(ones_t[:], ones_f[:])   # legal fp32r producer
            ones_r = ones_t[:]

            s_part = const_pool.tile([128, NS], dt.float32)
            # Row-sums: ones[128,128] lhsT replicates each chunk's sums
            # across all 128 partitions (PE PSUM writes must start at
            # partition 0 for fp32r). Each step's chunks land in a fresh
            # 2-bank PSUM tile; one DVE add folds them into acc_sb.
            # Slot s of acc_sb covers free [512s, 512s+512).
            acc_sb = const_pool.tile([128, 4096], dt.float32)
            nc.gpsimd.memset(acc_sb[:], 0.0)

            # Deferred ones emission: the PE queue is in-order, so a ones
            # matmul right after its group's EXP stalls the PE on ACT
            # every group (keeping the HAM clock throttled). Queue each
            # block and emit it one group later, when its P is ready.
            pending_ones = []

            def flush_ones():
                if not pending_ones:
                    return
                pairs, acc_lo, acc_w = pending_ones.pop(0)
                po = po_pool.tile([128, G1], dt.float32, tag="po")
                off = 0
                for rhs, w in pairs:
                    nc.tensor.matmul(po[:, off:off + w], ones_r, rhs,
                                     start=True, stop=True)
                    off += w
                assert off == acc_w
                nc.vector.tensor_add(
                    acc_sb[:, acc_lo:acc_lo + acc_w],
                    acc_sb[:, acc_lo:acc_lo + acc_w], po[:, 0:acc_w])

            def ones_block(pairs, acc_lo, acc_w):
                pending_ones.append((pairs, acc_lo, acc_w))

            def group(lhs_et, rhs_t, r0, w, P, p_off, slot, lhs_oh=None,
                      oh_t=None, split=False, mw=None):
                """One PSUM group: logit (+ mask over [0:mw]) matmuls,
                then EXP with fused accum into s_part[slot]."""
                ps = ps_pool.tile([128, G1], dt.float32, tag="ps")
                mw = w if mw is None else mw
                off = 0
                while off < w:
                    cw = min(512, w - off)
                    nc.tensor.matmul(ps[:, off:off + cw], lhs_et,
                                     rhs_t[:, r0 + off:r0 + off + cw],
                                     start=True,
                                     stop=(lhs_oh is None or off >= mw))
                    off += cw
                flush_ones()       # previous group's P is ready by now
                if lhs_oh is not None:
                    off = 0
                    while off < mw:
                        cw = min(512, mw - off)
                        nc.tensor.matmul(ps[:, off:off + cw], lhs_oh,
                                         oh_t[:, r0 + off:r0 + off + cw],
                                         start=False, stop=True)
                        off += cw
                if split:
                    nc.scalar.activation(
                        P[:, p_off:p_off + FIRST], ps[:, 0:FIRST],
                        mybir.ActivationFunctionType.Exp,
                        scale=SCALE, accum_out=s_part[:, slot:slot + 1])
                    nc.scalar.activation(
                        P[:, p_off + FIRST:p_off + w], ps[:, FIRST:w],
                        mybir.ActivationFunctionType.Exp,
                        scale=SCALE, accum_out=s_part[:, slot + 1:slot + 2])
                else:
                    nc.scalar.activation(
                        P[:, p_off:p_off + w], ps[:, 0:w],
                        mybir.ActivationFunctionType.Exp,
                        scale=SCALE, accum_out=s_part[:, slot:slot + 1])

            def w_step(k):
                lhs_et = t_et_own[:, k * 128:(k + 1) * 128]
                lhs_oh = t_ohn_own[:, k * 128:(k + 1) * 128]
                sl = slot_of[f"w{k}"]
                P = p_pool.tile([128, WIN], dt.float32r, tag="P")
                # group W1: own-block rows [0:1024] — accum only
                group(lhs_et, t_et_win, 0, G1, P, 0, sl[0],
                      lhs_oh=lhs_oh, oh_t=t_oh_win)
                # group W2: FWD rows [1024:2048] — accum + ones; same-label
                # rows end < 1024a+1280, so mask only [1024:1536].
                group(lhs_et, t_et_win, G1, G1, P, G1, sl[1],
                      lhs_oh=lhs_oh, oh_t=t_oh_win, mw=512)
                ones_block([(P[:, 1024:1536], 512), (P[:, 1536:2048], 512)],
                           3072, 1024)
                # ship numerator rows
                q = nc.sync if k % 2 == 0 else nc.gpsimd
                q.dma_start(pout[:, k * LNW:(k + 1) * LNW],
                            P[:, 0:LNW])

            def m_step(k):
                lhs_et = t_et_own[:, k * 128:(k + 1) * 128]
                sl = slot_of[f"m{k}"]
                n_main = (17 + k) * 128
                n_ones = (16 + k) * 128
                # groups over band rows [0:1024), [1024:2048), [2048:n_main)
                Pa = d_pool.tile([128, G1], dt.float32r, tag="Pm")
                group(lhs_et, t_et_band, 0, G1, Pa, 0, sl[0],
                      split=(k == 0))
                si = 2 if k == 0 else 1
                ones_block([(Pa[:, 0:512], 512), (Pa[:, 512:1024], 512)],
                           0, 1024)
                Pb = d_pool.tile([128, G1], dt.float32r, tag="Pm")
                group(lhs_et, t_et_band, G1, G1, Pb, 0, sl[si])
                ones_block([(Pb[:, 0:512], 512), (Pb[:, 512:1024], 512)],
                           1024, 1024)
                wd = n_main - 2048
                Pc = d_pool.tile([128, G1], dt.float32r, tag="Pm")
                group(lhs_et, t_et_band, 2048, wd, Pc, 0, sl[si + 1])
                # ones over band rows [2048:n_ones] (width 128k)
                w3 = n_ones - 2048
                if w3 > 0:
                    pairs = [(Pc[:, 0:min(512, w3)], min(512, w3))]
                    if w3 > 512:
                        pairs.append((Pc[:, 512:w3], w3 - 512))
                    ones_block(pairs, 2048, w3)

            for step in ORDER:
                (m_step if step[0] == "m" else w_step)(int(step[1]))
            while pending_ones:
                flush_ones()

            nc.sync.dma_start(out[:], s_part[:])
            nc.gpsimd.dma_start(ones_out[:], acc_sb[0:1, :])
    nc.compile()
    return nc


def _get_nc():
    if "nc" not in _cache:
        _cache["nc"] = _build()
    return _cache["nc"]


def _prepare(embeds, labels):
    embeds = np.ascontiguousarray(np.asarray(embeds, dtype=np.float32))
    labels_i = np.asarray(labels).astype(np.int64)
    assert embeds.shape == (B, D)

    perm = np.argsort(labels_i, kind="stable")
    lab = labels_i[perm]
    emb = embeds[perm]

    et = np.ascontiguousarray(emb.T).astype(ml_dtypes.bfloat16)   # [D, B]
    oh = np.zeros((NCLS, B), dtype=ml_dtypes.bfloat16)
    oh[lab, np.arange(B)] = ml_dtypes.bfloat16(1.0)
    ohn = (oh.astype(np.float32) * np.float32(MASKVAL)).astype(
        ml_dtypes.bfloat16)

    in_maps = []
    for c in range(NCORES):
        lo = c * COLS
        win = (lo + np.arange(WIN)) % B
        band = (lo + WIN + np.arange(BAND)) % B
        r_hi = np.searchsorted(lab, lab[lo + COLS - 1], side="right")
        assert r_hi - lo <= LNW, f"window overflow: {r_hi - lo}"
        in_maps.append({
            "et_own": np.ascontiguousarray(et[:, lo:lo + COLS]),
            "et_win": np.ascontiguousarray(et[:, win]),
            "et_band": np.ascontiguousarray(et[:, band]),
            "oh_win": np.ascontiguousarray(oh[:, win]),
            "ohn_own": np.ascontiguousarray(ohn[:, lo:lo + COLS]),
        })
    return in_maps, lab


def _combine(results, lab):
    slot_of, NS = _slots()
    S = np.zeros(B, dtype=np.float64)
    for a in range(NCORES):
        o = results[a]["out"]
        oo = results[a]["ones_out"].astype(np.float64).reshape(-1)
        sp = o[:, 0:NS].astype(np.float64)
        # column accums: sum each col-tile's slots
        for k in range(NCT):
            cols = a * COLS + k * 128 + np.arange(128)
            acc = np.zeros(128)
            for st in (f"m{k}", f"w{k}"):
                for s in slot_of[st]:
                    acc += sp[:, s]
            S[cols] += acc
        # ones row-sums
        win = (a * COLS + np.arange(WIN)) % B
        band = (a * COLS + WIN + np.arange(BAND)) % B
        S[band] += oo[0:BAND]                   # main slots 0..5
        S[win[1024:2048]] += oo[3072:4096]      # window FWD slots 6,7

    counts = np.bincount(lab, minlength=NCLS)
    count_j = counts[lab].astype(np.float64) - 1.0
    logS = np.log(S)

    total = 0.0
    for a in range(NCORES):
        pw = results[a]["pout"].reshape(128, NCT, LNW)
        win = (a * COLS + np.arange(LNW)) % B
        lab_win = lab[win]
        for k in range(NCT):
            cols = a * COLS + k * 128 + np.arange(128)
            m = (lab[cols][:, None] == lab_win[None, :]) \
                & (win[None, :] > cols[:, None])
            pj, rj = np.nonzero(m)
            i_idx = cols[pj]
            j_idx = win[rj]
            l = np.log(pw[pj, k, rj].astype(np.float64)) + C_USED
            ce_ij = np.logaddexp(-EPS, logS[j_idx] - l)
            ce_ji = np.logaddexp(-EPS, logS[i_idx] - l)
            total += (ce_ij / count_j[j_idx]).sum() \
                + (ce_ji / count_j[i_idx]).sum()

    loss = total / B
    return np.asarray(loss, dtype=np.float32)


def kernel(embeds, labels):
    in_maps, lab = _prepare(embeds, labels)
    nc = _get_nc()
    res = run_bass_kernel_spmd(nc, in_maps, list(range(NCORES)))
    return _combine(res.results, lab)


# revision 37
# speedup vs baseline: 1.4427x; 1.1382x over previous
"""EpsSupInfoNCE loss on 8 Trainium2 NeuronCores — symmetry-dedup version.

logits = (E@E.T)/temp is SYMMETRIC, so each off-diagonal exp is computed
ONCE device-wide and feeds BOTH sums it belongs to: S for its column via
the ACT fused per-instruction accumulator, and S for its row via a
ones-vector matmul over P on the (underutilized) tensor engine.

Layout (host sorts all rows/cols by label first):
- core a owns cols [1024a, 1024(a+1)); 8 col-tiles of 128.
- window = rows [1024a, 1024a+2048) mod B  (own block + next block),
  MASKED via one-hot matmul (-4.5 in dot units -> -C in logit units).
  Own-block rows feed column-accum only (each within-block pair appears
  twice, once per mirror entry -> once per S side). Next-block rows feed
  column-accum AND a ones-matmul row-sum.
- main = sliding prefix of a circular band band[x] = 1024a+2048+x,
  x < 3072: col-tile k computes band[0:(17+k)*128]; rows [0:(16+k)*128]
  feed accum+ones; the last 128 rows (= tile u+32 for col-tile u) feed
  accum ONLY — its mirror tile does the same, covering diff-32 pairs
  once per side. All other cross-block tile pairs {u, v} are covered
  exactly once by the circular tournament v in {u+1..u+31}.
- numerator: P_win[:, 0:1280] shipped raw to DRAM; host recovers
  l = ln(P)+C at same-label entries (upper triangle, row>col) and
  computes both ordered ce terms from the single symmetric value.
- ones row-sums accumulate in persistent PSUM strips (2 banks, 8 slots
  of [1,512] at partitions 0/32/64/96) across all col-tiles; one DMA
  per slot at the end. PSUM: 2x 3-bank matmul groups + 2 banks = 8.
"""
import numpy as np
import ml_dtypes
from contextlib import ExitStack

import concourse.bacc as bacc
import concourse.tile as tile
from concourse import mybir
from concourse.bass_utils import run_bass_kernel_spmd

B, D = 8192, 128
NCLS = 100
NCORES = 8
COLS = B // NCORES            # 1024 columns per core
NCT = COLS // 128             # 8 col-tiles per core
WIN = 2048                    # window rows per core (blocks a, a+1)
BAND = 3072                   # circular main band rows per core
LNW = 1280                    # shipped window rows (numerator span)
G1 = 1024                     # PSUM group width (2 banks; ones need 3,
                              # PE PSUM col base is limited to {0,32,64})
FIRST = 512                   # m0's tiny first chunk (early ACT start)

TEMP = 0.07
EPS = 0.25
SCALE = float(np.float32(1.0) / np.float32(TEMP))   # exp scale (fp32 value)
MASKVAL = -4.5                                      # bf16-exact additive mask
C_USED = 4.5 * SCALE                                # mask size in logit units

# program order: window k ships 640KB of P, so keep the last mains after
# the last window to hide the final ship under trailing ACT work.
ORDER = ["m0", "m1", "m2", "w0", "m3", "w1", "m4", "w2", "m5", "w3",
         "w4", "w5", "w6", "w7", "m6", "m7"]

_cache = {}


def _patch_act_tables():
    """Steer Exp onto a stable table set (baseline's patch; harmless now
    that only Exp is used)."""
    import concourse.hw_specs as hw_specs
    from concourse import mybir as _mb
    if getattr(bacc, "_act_tables_patched", False):
        return
    orig = hw_specs.get_activation_tables

    def steer(arch):
        t = orig(arch)
        exp, ln = (_mb.ActivationFunctionType.Exp, _mb.ActivationFunctionType.Ln)
        if "natural_log_exp_and_others" not in t:
            return t
        return {k: (fns if k == "natural_log_exp_and_others"
                    else fns - {exp, ln}) for k, fns in t.items()}

    bacc.get_activation_tables = steer
    bacc._act_tables_patched = True


def _slots():
    """Deterministic accum-slot layout shared by _build and _combine.
    Returns (slot_of[step_name] -> list of slot ids, NS)."""
    slot_of = {}
    n = 0
    for step in ORDER:
        if step[0] == "m":
            nslots = 3 + (1 if step == "m0" else 0)
        else:
            nslots = 2
        slot_of[step] = list(range(n, n + nslots))
        n += nslots
    return slot_of, n


def _build():
    dt = mybir.dt
    _patch_act_tables()
    nc = bacc.Bacc("TRN2", target_bir_lowering=False, debug=False,
                   num_devices=NCORES)
    et_own = nc.dram_tensor("et_own", [D, COLS], dt.bfloat16,
                            kind="ExternalInput").ap()
    et_win = nc.dram_tensor("et_win", [D, WIN], dt.bfloat16,
                            kind="ExternalInput").ap()
    et_band = nc.dram_tensor("et_band", [D, BAND], dt.bfloat16,
                             kind="ExternalInput").ap()
    oh_win = nc.dram_tensor("oh_win", [NCLS, WIN], dt.bfloat16,
                            kind="ExternalInput").ap()
    ohn_own = nc.dram_tensor("ohn_own", [NCLS, COLS], dt.bfloat16,
                             kind="ExternalInput").ap()
    slot_of, NS = _slots()
    out = nc.dram_tensor("out", [128, NS], dt.float32,
                         kind="ExternalOutput").ap()
    # P is produced as float32r (PE consumes it as fp32r moving data);
    # same bits as f32 on the host side.
    pout = nc.dram_tensor("pout", [128, NCT * LNW], dt.float32r,
                          kind="ExternalOutput").ap()
    ones_out = nc.dram_tensor("ones_out", [1, 4096], dt.float32,
                              kind="ExternalOutput").ap()

    with tile.TileContext(nc) as tc:
        with ExitStack() as ctx:
            const_pool = ctx.enter_context(tc.tile_pool(name="consts", bufs=1))
            p_pool = ctx.enter_context(tc.tile_pool(name="pwin", bufs=3))
            d_pool = ctx.enter_context(tc.tile_pool(name="pmain", bufs=5))
            ps_pool = ctx.enter_context(
                tc.tile_pool(name="psum", bufs=2, space="PSUM"))
            po_pool = ctx.enter_context(
                tc.tile_pool(name="psones", bufs=2, space="PSUM"))

            t_et_own = const_pool.tile([D, COLS], dt.bfloat16)
            t_et_win = const_pool.tile([D, WIN], dt.bfloat16)
            t_et_band = const_pool.tile([D, BAND], dt.bfloat16)
            t_oh_win = const_pool.tile([NCLS, WIN], dt.bfloat16)
            t_ohn_own = const_pool.tile([NCLS, COLS], dt.bfloat16)
            # Inputs: head on sync (first exp waits ~160KB), bulk split
            # so the 5.2MB of P ships own the sync hw queue afterwards.
            # oh_win[1536:] is never read (mask is trimmed past 1536).
            nc.sync.dma_start(t_et_own[:, 0:128], et_own[:, 0:128])
            nc.sync.dma_start(t_et_band[:, 0:FIRST], et_band[:, 0:FIRST])
            nc.sync.dma_start(t_et_band[:, FIRST:G1], et_band[:, FIRST:G1])
            nc.gpsimd.dma_start(t_et_band[:, G1:2432], et_band[:, G1:2432])
            nc.sync.dma_start(t_et_win[:, 0:G1], et_win[:, 0:G1])
            nc.sync.dma_start(t_oh_win[:, 0:1536], oh_win[:, 0:1536])
            nc.sync.dma_start(t_ohn_own[:, 0:128], ohn_own[:, 0:128])
            nc.gpsimd.dma_start(t_et_win[:, G1:], et_win[:, G1:])
            nc.sync.dma_start(t_et_own[:, 128:], et_own[:, 128:])
            nc.gpsimd.dma_start(t_et_band[:, 2432:], et_band[:, 2432:])
            nc.gpsimd.dma_start(t_ohn_own[:, 128:], ohn_own[:, 128:])

            ones_f = const_pool.tile([128, 128], dt.float32)
            nc.gpsimd.memset(ones_f[:], 1.0)
            ones_t = const_pool.tile([128, 128], dt.float32r)
            nc.scalar.copy(ones_t[:], ones_f[:])   # legal fp32r producer
            ones_r = ones_t[:]

            s_part = const_pool.tile([128, NS], dt.float32)
            # Row-sums: ones[128,128] lhsT replicates each chunk's sums
            # across all 128 partitions (PE PSUM writes must start at
            # partition 0 for fp32r). Each step's chunks land in a fresh
            # 2-bank PSUM tile; one DVE add folds them into acc_sb.
            # Slot s of acc_sb covers free [512s, 512s+512).
            acc_sb = const_pool.tile([128, 4096], dt.float32)
            nc.gpsimd.memset(acc_sb[:], 0.0)

            # Deferred ones emission: the PE queue is in-order, so a ones
            # matmul right after its group's EXP stalls the PE on ACT
            # every group (keeping the HAM clock throttled). Queue each
            # block and emit it one group later, when its P is ready.
            pending_ones = []

            def flush_ones(keep=2):
                # depth-2 deferral: EXP(i)+readout outlast group i+1's
                # matmuls, so emit ones(i) during group i+2.
                if len(pending_ones) <= keep:
                    return
                pairs, acc_lo, acc_w = pending_ones.pop(0)
                po = po_pool.tile([128, G1], dt.float32, tag="po")
                off = 0
                for rhs, w in pairs:
                    nc.tensor.matmul(po[:, off:off + w], ones_r, rhs,
                                     start=True, stop=True)
                    off += w
                assert off == acc_w
                nc.vector.tensor_add(
                    acc_sb[:, acc_lo:acc_lo + acc_w],
                    acc_sb[:, acc_lo:acc_lo + acc_w], po[:, 0:acc_w])

            def ones_block(pairs, acc_lo, acc_w):
                pending_ones.append((pairs, acc_lo, acc_w))

            def group(lhs_et, rhs_t, r0, w, P, p_off, slot, lhs_oh=None,
                      oh_t=None, split=False, mw=None):
                """One PSUM group: logit (+ mask over [0:mw]) matmuls,
                then EXP with fused accum into s_part[slot]."""
                ps = ps_pool.tile([128, G1], dt.float32, tag="ps")
                mw = w if mw is None else mw
                off = 0
                while off < w:
                    cw = min(512, w - off)
                    nc.tensor.matmul(ps[:, off:off + cw], lhs_et,
                                     rhs_t[:, r0 + off:r0 + off + cw],
                                     start=True,
                                     stop=(lhs_oh is None or off >= mw))
                    off += cw
                flush_ones()       # previous group's P is ready by now
                if lhs_oh is not None:
                    off = 0
                    while off < mw:
                        cw = min(512, mw - off)
                        nc.tensor.matmul(ps[:, off:off + cw], lhs_oh,
                                         oh_t[:, r0 + off:r0 + off + cw],
                                         start=False, stop=True)
                        off += cw
                if split:
                    nc.scalar.activation(
                        P[:, p_off:p_off + FIRST], ps[:, 0:FIRST],
                        mybir.ActivationFunctionType.Exp,
                        scale=SCALE, accum_out=s_part[:, slot:slot + 1])
                    nc.scalar.activation(
                        P[:, p_off + FIRST:p_off + w], ps[:, FIRST:w],
                        mybir.ActivationFunctionType.Exp,
                        scale=SCALE, accum_out=s_part[:, slot + 1:slot + 2])
                else:
                    nc.scalar.activation(
                        P[:, p_off:p_off + w], ps[:, 0:w],
                        mybir.ActivationFunctionType.Exp,
                        scale=SCALE, accum_out=s_part[:, slot:slot + 1])

            def w_step(k):
                lhs_et = t_et_own[:, k * 128:(k + 1) * 128]
                lhs_oh = t_ohn_own[:, k * 128:(k + 1) * 128]
                sl = slot_of[f"w{k}"]
                P = p_pool.tile([128, WIN], dt.float32r, tag="P")
                # group W1: own-block rows [0:1024] — accum only
                group(lhs_et, t_et_win, 0, G1, P, 0, sl[0],
                      lhs_oh=lhs_oh, oh_t=t_oh_win)
                # group W2: FWD rows [1024:2048] — accum + ones; same-label
                # rows end < 1024a+1280, so mask only [1024:1536].
                group(lhs_et, t_et_win, G1, G1, P, G1, sl[1],
                      lhs_oh=lhs_oh, oh_t=t_oh_win, mw=512)
                ones_block([(P[:, 1024:1536], 512), (P[:, 1536:2048], 512)],
                           3072, 1024)
                # ship numerator rows (sync = hardware DGE queue)
                nc.sync.dma_start(pout[:, k * LNW:(k + 1) * LNW],
                                  P[:, 0:LNW])

            def m_step(k):
                lhs_et = t_et_own[:, k * 128:(k + 1) * 128]
                sl = slot_of[f"m{k}"]
                n_main = (17 + k) * 128
                n_ones = (16 + k) * 128
                # groups over band rows [0:1024), [1024:2048), [2048:n_main)
                Pa = d_pool.tile([128, G1], dt.float32r, tag="Pm")
                group(lhs_et, t_et_band, 0, G1, Pa, 0, sl[0],
                      split=(k == 0))
                si = 2 if k == 0 else 1
                ones_block([(Pa[:, 0:512], 512), (Pa[:, 512:1024], 512)],
                           0, 1024)
                Pb = d_pool.tile([128, G1], dt.float32r, tag="Pm")
                group(lhs_et, t_et_band, G1, G1, Pb, 0, sl[si])
                ones_block([(Pb[:, 0:512], 512), (Pb[:, 512:1024], 512)],
                           1024, 1024)
                wd = n_main - 2048
                Pc = d_pool.tile([128, G1], dt.float32r, tag="Pm")
                group(lhs_et, t_et_band, 2048, wd, Pc, 0, sl[si + 1])
                # ones over band rows [2048:n_ones] (width 128k)
                w3 = n_ones - 2048
                if w3 > 0:
                    pairs = [(Pc[:, 0:min(512, w3)], min(512, w3))]
                    if w3 > 512:
                        pairs.append((Pc[:, 512:w3], w3 - 512))
                    ones_block(pairs, 2048, w3)

            for step in ORDER:
                (m_step if step[0] == "m" else w_step)(int(step[1]))
            while pending_ones:
                flush_ones(keep=0)

            nc.sync.dma_start(out[:], s_part[:])
            nc.gpsimd.dma_start(ones_out[:], acc_sb[0:1, :])
    nc.compile()
    return nc


def _get_nc():
    if "nc" not in _cache:
        _cache["nc"] = _build()
    return _cache["nc"]


def _prepare(embeds, labels):
    embeds = np.ascontiguousarray(np.asarray(embeds, dtype=np.float32))
    labels_i = np.asarray(labels).astype(np.int64)
    assert embeds.shape == (B, D)

    perm = np.argsort(labels_i, kind="stable")
    lab = labels_i[perm]
    emb = embeds[perm]

    et = np.ascontiguousarray(emb.T).astype(ml_dtypes.bfloat16)   # [D, B]
    oh = np.zeros((NCLS, B), dtype=ml_dtypes.bfloat16)
    oh[lab, np.arange(B)] = ml_dtypes.bfloat16(1.0)
    ohn = (oh.astype(np.float32) * np.float32(MASKVAL)).astype(
        ml_dtypes.bfloat16)

    in_maps = []
    for c in range(NCORES):
        lo = c * COLS
        win = (lo + np.arange(WIN)) % B
        band = (lo + WIN + np.arange(BAND)) % B
        r_hi = np.searchsorted(lab, lab[lo + COLS - 1], side="right")
        assert r_hi - lo <= LNW, f"window overflow: {r_hi - lo}"
        in_maps.append({
            "et_own": np.ascontiguousarray(et[:, lo:lo + COLS]),
            "et_win": np.ascontiguousarray(et[:, win]),
            "et_band": np.ascontiguousarray(et[:, band]),
            "oh_win": np.ascontiguousarray(oh[:, win]),
            "ohn_own": np.ascontiguousarray(ohn[:, lo:lo + COLS]),
        })
    return in_maps, lab


def _combine(results, lab):
    slot_of, NS = _slots()
    S = np.zeros(B, dtype=np.float64)
    for a in range(NCORES):
        o = results[a]["out"]
        oo = results[a]["ones_out"].astype(np.float64).reshape(-1)
        sp = o[:, 0:NS].astype(np.float64)
        # column accums: sum each col-tile's slots
        for k in range(NCT):
            cols = a * COLS + k * 128 + np.arange(128)
            acc = np.zeros(128)
            for st in (f"m{k}", f"w{k}"):
                for s in slot_of[st]:
                    acc += sp[:, s]
            S[cols] += acc
        # ones row-sums
        win = (a * COLS + np.arange(WIN)) % B
        band = (a * COLS + WIN + np.arange(BAND)) % B
        S[band] += oo[0:BAND]                   # main slots 0..5
        S[win[1024:2048]] += oo[3072:4096]      # window FWD slots 6,7

    counts = np.bincount(lab, minlength=NCLS)
    count_j = counts[lab].astype(np.float64) - 1.0
    logS = np.log(S)

    total = 0.0
    for a in range(NCORES):
        pw = results[a]["pout"].reshape(128, NCT, LNW)
        win = (a * COLS + np.arange(LNW)) % B
        lab_win = lab[win]
        for k in range(NCT):
            cols = a * COLS + k * 128 + np.arange(128)
            m = (lab[cols][:, None] == lab_win[None, :]) \
                & (win[None, :] > cols[:, None])
            pj, rj = np.nonzero(m)
            i_idx = cols[pj]
            j_idx = win[rj]
            l = np.log(pw[pj, k, rj].astype(np.float64)) + C_USED
            ce_ij = np.logaddexp(-EPS, logS[j_idx] - l)
            ce_ji = np.logaddexp(-EPS, logS[i_idx] - l)
            total += (ce_ij / count_j[j_idx]).sum() \
                + (ce_ji / count_j[i_idx]).sum()

    loss = total / B
    return np.asarray(loss, dtype=np.float32)


def kernel(embeds, labels):
    in_maps, lab = _prepare(embeds, labels)
    nc = _get_nc()
    res = run_bass_kernel_spmd(nc, in_maps, list(range(NCORES)))
    return _combine(res.results, lab)
